# revision 40
# baseline (speedup 1.0000x reference)
"""Causal attention head (RoPE) kernel for 8 Trainium2 NeuronCores.

Sharding: 8 cores = 2 batches x 4 head-groups (4 heads each), no
cross-device comms. Per-core design (v12):

  - ONE bf16 x load (4 per-512-t-chunk tiles, gpsimd DMA ring); the fp8
    copy for the q/k projections is cast chunk-by-chunk on the ACT engine
    (idle before softmax), so the fp8 path starts as each chunk lands.
  - Q/K projections as fp8 DoubleRow matmuls (K_eff=256/instr); RoPE per
    1024-col half: cos/sin products on DVE (bf16), partition swap via 4
    sync-ring DMAs, one wide DVE sub -> roped bf16 half-tiles. Scores over
    the first q/k half can start while the second half is still roping.
  - V projected straight into natural (key-major) layout with x^T bf16
    stationary (no PE transposes); ones column per head makes PV row 64
    the softmax denominator. fp8 anywhere in the P/V path provably
    exceeds the error budget, so it stays bf16.
  - attention is qc-major (per 512-query chunk, all key blocks up to the
    diagonal), scores bf16 dual-tile (two heads on PE row-tiles 0-63 /
    64-127, K=64). Score PSUM tiles hold PAIRS of key blocks so exp runs
    >=384 wide (exp has ~700ns fixed cost); diagonal blocks use
    partial-width matmuls. PV for chunk qc is issued during chunk qc+1's
    scores so the PE never waits on exp (PE p-state: any gap halves the
    clock); pair-1 projections and V-proj fill remaining PE slack.
  - po [65, 512] PSUM -> DVE copy -> HBM unnormalized; the host divides
    by the denominator row on gather (free).
"""

import os
import sys
from contextlib import ExitStack

import numpy as np

for _p in ("/opt/trn_rl_repo", "/root/.axon_site/_ro/trn_rl_repo"):
    if os.path.isdir(_p) and _p not in sys.path:
        sys.path.append(_p)

import ml_dtypes

import concourse.bass as bass
import concourse.mybir as mybir
import concourse.tile as tile
from concourse import bacc
from concourse.bass_utils import run_bass_kernel_spmd

P = 128
T = 2048
CIN = 1024
NHC = 4          # heads per core
HS = 64
DOUT = NHC * HS  # 256
NCT = CIN // P   # 8 contraction tiles
NCP = NCT // 2   # 4 fp8 DoubleRow contraction pairs
SCALE = 1.0 / 32.0  # 1024 ** -0.5

F32 = mybir.dt.float32
BF16 = mybir.dt.bfloat16
F8 = mybir.dt.float8e4
DR = mybir.MatmulPerfMode.DoubleRow


def _build_nc():
    nc = bacc.Bacc("TRN2")

    xbT = nc.dram_tensor("xbT", [CIN, T], BF16, kind="ExternalInput").ap()
    wq8 = nc.dram_tensor("wq8", [CIN, DOUT], F8, kind="ExternalInput").ap()
    wk8 = nc.dram_tensor("wk8", [CIN, DOUT], F8, kind="ExternalInput").ap()
    wvT = nc.dram_tensor("wvT", [CIN, DOUT], BF16, kind="ExternalInput").ap()
    cos4 = nc.dram_tensor("cos4", [P, T], BF16, kind="ExternalInput").ap()
    sin4 = nc.dram_tensor("sin4", [P, T], BF16, kind="ExternalInput").ap()
    utri = nc.dram_tensor("utri", [P, P], BF16, kind="ExternalInput").ap()
    outT = nc.dram_tensor("outT", [NHC * (HS + 1), T], F32, kind="ExternalOutput").ap()

    with tile.TileContext(nc) as tc, ExitStack() as ctx:
        const_pool = ctx.enter_context(tc.tile_pool(name="const", bufs=1))
        wpool = ctx.enter_context(tc.tile_pool(name="w", bufs=1))
        xpool = ctx.enter_context(tc.tile_pool(name="x", bufs=1))
        qkpool = ctx.enter_context(tc.tile_pool(name="qk", bufs=1))
        mpool = ctx.enter_context(tc.tile_pool(name="m", bufs=2))
        vpool = ctx.enter_context(tc.tile_pool(name="vaug", bufs=1))
        ppool = ctx.enter_context(tc.tile_pool(name="pt", bufs=2))
        opool = ctx.enter_context(tc.tile_pool(name="ob", bufs=3))

        pp_acc = ctx.enter_context(tc.tile_pool(name="pp_acc", bufs=2, space="PSUM"))
        pp_s = ctx.enter_context(tc.tile_pool(name="pp_s", bufs=2, space="PSUM"))
        pp_po = ctx.enter_context(tc.tile_pool(name="pp_po", bufs=2, space="PSUM"))

        # ---- input DMAs. sync ring: trig first (rope needs it early), then
        # weights, then the rope swap DMAs, then outputs. gpsimd ring: the
        # four x chunks. scalar ring carries only casts+exps (any DMA there
        # would head-of-line-block softmax).
        cos_s = const_pool.tile([P, T], BF16, tag="cos")
        nc.sync.dma_start(cos_s[:], cos4)
        sin_s = const_pool.tile([P, T], BF16, tag="sin")
        nc.sync.dma_start(sin_s[:], sin4)
        w_tiles = {}
        for name, wsrc, dt in (("q", wq8, F8), ("k", wk8, F8), ("v", wvT, BF16)):
            w_s = wpool.tile([P, NCT * DOUT], dt, tag=f"w{name}", name=f"w{name}")
            nc.sync.dma_start(
                w_s.rearrange("p (n d) -> p n d", n=NCT),
                wsrc.rearrange("(n p) d -> p n d", p=P),
            )
            w_tiles[name] = w_s
        utri_s = const_pool.tile([P, P], BF16, tag="utri")
        nc.sync.dma_start(utri_s[:], utri)
        xb_r = xbT.rearrange("(n p) t -> p n t", p=P)
        xsb_t, xs8_t = [], []
        for ch in range(4):
            xt = xpool.tile([P, NCT * 512], BF16, tag=f"xsb{ch}", name=f"xsb{ch}")
            nc.gpsimd.dma_start(
                xt.rearrange("p (n t) -> p n t", n=NCT),
                xb_r[:, :, ch * 512:(ch + 1) * 512],
            )
            xsb_t.append(xt)
            x8 = xpool.tile([P, NCT * 512], F8, tag=f"xs8{ch}", name=f"xs8{ch}")
            nc.scalar.copy(x8[:], xt[:])
            xs8_t.append(x8)

        # roped q/k as per-1024-col half tiles (head pair m: rows 0-63/64-127)
        qth = [[qkpool.tile([P, 1024], BF16, tag=f"qt{m}_{h2}", name=f"qt{m}_{h2}")
                for h2 in range(2)] for m in range(2)]
        kth = [[qkpool.tile([P, 1024], BF16, tag=f"kt{m}_{h2}", name=f"kt{m}_{h2}")
                for h2 in range(2)] for m in range(2)]
        va = [
            vpool.tile([P, NHC * (HS + 1)], BF16, tag=f"vaug{tb}", name=f"vaug{tb}")
            for tb in range(T // P)
        ]

        def proj_rope(m, wname, dsts):
            """fp8 DR projection of an m-tile (2 heads) + RoPE per half."""
            w_r = w_tiles[wname].rearrange("p (n d) -> p n d", n=NCT)
            for half in range(2):
                ra = mpool.tile([P, 1024], BF16, tag="ra", name=f"ra{wname}{m}{half}")
                rp = mpool.tile([P, 1024], BF16, tag="rp", name=f"rp{wname}{m}{half}")
                for chh in range(2):
                    ch = half * 2 + chh
                    cs = slice(ch * 512, (ch + 1) * 512)
                    hs = slice(chh * 512, (chh + 1) * 512)
                    x8_r = xs8_t[ch].rearrange("p (n t) -> p n t", n=NCT)
                    ps = pp_acc.tile([P, 512], F32, tag="acc", name=f"pj{wname}{m}{ch}")
                    for cp in range(NCP):
                        nc.tensor.matmul(
                            ps[:],
                            lhsT=w_r[:, 2 * cp:2 * cp + 2, m * P:(m + 1) * P],
                            rhs=x8_r[:, 2 * cp:2 * cp + 2, :],
                            perf_mode=DR,
                            start=(cp == 0),
                            stop=(cp == NCP - 1),
                        )
                    nc.vector.tensor_mul(ra[:, hs], ps[:], cos_s[:, cs])
                    nc.vector.tensor_mul(rp[:, hs], ps[:], sin_s[:, cs])
                sw = mpool.tile([P, 1024], BF16, tag="rs", name=f"rs{wname}{m}{half}")
                for blk in range(4):
                    s0 = (blk ^ 1) * 32
                    nc.sync.dma_start(
                        sw[blk * 32:(blk + 1) * 32, :], rp[s0:s0 + 32, :]
                    )
                nc.vector.tensor_sub(dsts[half][:], ra[:], sw[:])

        def vproj(tbp):
            """bf16 V proj of t-blocks (2*tbp, 2*tbp+1) into natural layout."""
            pv = pp_acc.tile([P, 512], F32, tag="acc", name=f"pv{tbp}")
            wv_r = w_tiles["v"].rearrange("p (n d) -> p n d", n=NCT)
            for i in range(2):
                tb = 2 * tbp + i
                xb_c = xsb_t[tb // 4].rearrange("p (n t) -> p n t", n=NCT)
                tb4 = tb % 4
                for c in range(NCT):
                    nc.tensor.matmul(
                        pv[:, i * DOUT:(i + 1) * DOUT],
                        lhsT=xb_c[:, c, tb4 * P:(tb4 + 1) * P],
                        rhs=wv_r[:, c, :],
                        start=(c == 0),
                        stop=(c == NCT - 1),
                        skip_group_check=True,
                    )
            pv_r = pv.rearrange("p (i h d) -> p i h d", i=2, h=NHC)
            for i in range(2):
                vt_r = va[2 * tbp + i].rearrange("p (h e) -> p h e", e=HS + 1)
                nc.gpsimd.memset(vt_r[:, :, HS:HS + 1], 1.0)
                nc.vector.tensor_copy(vt_r[:, :, 0:HS], pv_r[:, i, :, :])

        def scores_head(m, hi, qc):
            """scores+exp+mask for head (m,hi), query chunk qc.

            Key blocks are processed in pairs sharing one [128,1024] PSUM
            tile so each exp instruction covers both blocks' valid columns
            (contiguous by construction). Returns the PV plan:
            [(j, pt, ptoff, width, col0), ...] in ascending j."""
            r0 = hi * HS
            q0 = qc * 512
            qt_h = qth[m][q0 // 1024]
            qq0 = q0 % 1024
            plan = []
            for jp in range(2 * qc + 2):
                ps = pp_s.tile([P, 1024], F32, tag="ps", name=f"ps{m}{hi}{qc}{jp}")
                pt = ppool.tile([P, 1024], BF16, tag=f"pt{hi}_{jp}",
                                name=f"pt{m}{hi}{qc}{jp}")
                off = 0
                for j in (2 * jp, 2 * jp + 1):
                    col0 = max(0, j * P - q0)
                    w = 512 - col0
                    kt_h = kth[m][(j * P) // 1024]
                    kk0 = (j * P) % 1024
                    nc.tensor.matmul(
                        ps[:, off:off + w],
                        lhsT=kt_h[r0:r0 + HS, kk0:kk0 + P],
                        rhs=qt_h[r0:r0 + HS, qq0 + col0:qq0 + 512],
                        start=True,
                        stop=True,
                        tile_position=(r0, 0),
                    )
                    plan.append((j, pt, off, w, col0))
                    off += w
                nc.scalar.activation(
                    pt[:, 0:off], ps[:, 0:off],
                    mybir.ActivationFunctionType.Exp, scale=SCALE,
                )
            for j, pt, ptoff, w, col0 in plan:
                if col0 > 0 or j * P == q0:  # diagonal block: causal mask
                    nc.vector.tensor_mul(
                        pt[:, ptoff:ptoff + P], pt[:, ptoff:ptoff + P], utri_s[:]
                    )
            return plan

        def pv_head(m, hi, qc, plan):
            """PV accumulation + copy + output DMA for one head, 512-q chunk."""
            h = 2 * m + hi
            q0 = qc * 512
            po = pp_po.tile([HS + 1, 512], F32, tag="po", name=f"po{h}_{qc}")
            for i, (j, pt, ptoff, w, col0) in enumerate(plan):
                nc.tensor.matmul(
                    po[:, col0:512],
                    lhsT=va[j][:, h * (HS + 1):(h + 1) * (HS + 1)],
                    rhs=pt[:, ptoff:ptoff + w],
                    start=(i == 0),
                    stop=(i == len(plan) - 1),
                    skip_group_check=True,
                )
            ob = opool.tile([HS + 1, 512], F32, tag="ob", name=f"ob{h}_{qc}")
            nc.vector.tensor_copy(ob[:], po[:])
            nc.sync.dma_start(
                outT[h * (HS + 1):(h + 1) * (HS + 1), q0:q0 + 512], ob[:]
            )

        # ---- pair-0 projections, then qc-major attention per pair with
        # PV lagged one query chunk; pair-1 projections and V-proj are
        # issued inside pair-0's loop as PE filler.
        proj_rope(0, "q", qth[0])
        proj_rope(0, "k", kth[0])

        for m in (0, 1):
            plans = {}
            for qc in range(5):
                for hi in range(2):
                    if qc < 4:
                        sp = scores_head(m, hi, qc)
                    if qc >= 1:
                        pv_head(m, hi, qc - 1, plans[hi])
                    if qc < 4:
                        plans[hi] = sp
                if m == 0:
                    if qc == 0:
                        vproj(0)
                        vproj(1)
                        proj_rope(1, "q", qth[1])
                    elif qc == 1:
                        vproj(2)
                        vproj(3)
                        proj_rope(1, "k", kth[1])
                    elif qc == 2:
                        for tbp in (4, 5, 6, 7):
                            vproj(tbp)

    nc.compile()
    return nc


_CACHE = {}


def _get_nc():
    if "nc" not in _CACHE:
        _CACHE["nc"] = _build_nc()
    return _CACHE["nc"]


def _host_inputs(x, Wq, Wk, Wv):
    bf = ml_dtypes.bfloat16
    f8 = ml_dtypes.float8_e4m3
    B = x.shape[0]
    # RoPE tables (match reference: theta over hs/2 freqs with dim=n_emb)
    i = np.arange(HS // 2, dtype=np.float32)
    theta = np.float32(10000.0) ** (-2.0 * i / np.float32(CIN))
    pos = np.arange(T, dtype=np.float32)
    ang = pos[:, None] * theta[None, :]
    cosT = np.cos(ang).T.astype(np.float32)  # [32, T]
    sinT = np.sin(ang).T.astype(np.float32)
    cos4 = np.ascontiguousarray(np.tile(cosT, (4, 1))).astype(bf)
    sin4 = np.ascontiguousarray(
        np.tile(np.concatenate([-sinT, sinT], axis=0), (2, 1))
    ).astype(bf)  # rows: [-sin, +sin] x2
    utri_np = np.triu(np.ones((P, P), np.float32)).astype(bf)

    perm = np.concatenate([np.arange(0, HS, 2), np.arange(1, HS, 2)])
    in_maps = []
    for core in range(8):
        b, g = core // 4, core % 4
        idx = np.concatenate([(4 * g + h) * HS + perm for h in range(NHC)])
        xT = np.ascontiguousarray(x[b].T)
        m = {
            "xbT": xT.astype(bf),
            "wq8": np.ascontiguousarray(Wq[idx].T).astype(f8),
            "wk8": np.ascontiguousarray(Wk[idx].T).astype(f8),
            "wvT": np.ascontiguousarray(Wv[g * DOUT:(g + 1) * DOUT].T).astype(bf),
            "cos4": cos4,
            "sin4": sin4,
            "utri": utri_np,
        }
        in_maps.append(m)
    return in_maps


def kernel(x, Wq, Wk, Wv, _trace=False, _trace_kwargs=None):
    x = np.asarray(x)
    Wq, Wk, Wv = np.asarray(Wq), np.asarray(Wk), np.asarray(Wv)
    B = x.shape[0]
    nc = _get_nc()
    in_maps = _host_inputs(x, Wq, Wk, Wv)
    res = run_bass_kernel_spmd(
        nc, in_maps, list(range(8)), trace=_trace, **(_trace_kwargs or {})
    )
    out = np.zeros((B, T, CIN), np.float32)
    for core in range(8):
        b, g = core // 4, core % 4
        r = res.results[core]["outT"].reshape(NHC, HS + 1, T)
        o = r[:, 0:HS, :] / r[:, HS:HS + 1, :]
        out[b, :, g * DOUT:(g + 1) * DOUT] = o.reshape(DOUT, T).T
    if _trace:
        return out, res
    return out


# revision 43
# speedup vs baseline: 1.0302x; 1.0302x over previous
"""Causal attention head (RoPE) kernel for 8 Trainium2 NeuronCores.

Sharding: 8 cores = 2 batches x 4 head-groups (4 heads each), no
cross-device comms. Per-core design (v12):

  - ONE bf16 x load (4 per-512-t-chunk tiles, gpsimd DMA ring); the fp8
    copy for the q/k projections is cast chunk-by-chunk on the ACT engine
    (idle before softmax), so the fp8 path starts as each chunk lands.
  - Q/K projections as fp8 DoubleRow matmuls (K_eff=256/instr); RoPE per
    1024-col half: cos/sin products on DVE (bf16), partition swap via 4
    sync-ring DMAs, one wide DVE sub -> roped bf16 half-tiles. Scores over
    the first q/k half can start while the second half is still roping.
  - V projected straight into natural (key-major) layout with x^T bf16
    stationary (no PE transposes); ones column per head makes PV row 64
    the softmax denominator. fp8 anywhere in the P/V path provably
    exceeds the error budget, so it stays bf16.
  - attention is qc-major (per 512-query chunk, all key blocks up to the
    diagonal), scores bf16 dual-tile (two heads on PE row-tiles 0-63 /
    64-127, K=64). Score PSUM tiles hold PAIRS of key blocks so exp runs
    >=384 wide (exp has ~700ns fixed cost); diagonal blocks use
    partial-width matmuls. PV for chunk qc is issued during chunk qc+1's
    scores so the PE never waits on exp (PE p-state: any gap halves the
    clock); pair-1 projections and V-proj fill remaining PE slack.
  - po [65, 512] PSUM -> DVE copy -> HBM unnormalized; the host divides
    by the denominator row on gather (free).
"""

import os
import sys
from contextlib import ExitStack

import numpy as np

for _p in ("/opt/trn_rl_repo", "/root/.axon_site/_ro/trn_rl_repo"):
    if os.path.isdir(_p) and _p not in sys.path:
        sys.path.append(_p)

import ml_dtypes

import concourse.bass as bass
import concourse.mybir as mybir
import concourse.tile as tile
from concourse import bacc
from concourse.bass_utils import run_bass_kernel_spmd

P = 128
T = 2048
CIN = 1024
NHC = 4          # heads per core
HS = 64
DOUT = NHC * HS  # 256
NCT = CIN // P   # 8 contraction tiles
NCP = NCT // 2   # 4 fp8 DoubleRow contraction pairs
SCALE = 1.0 / 32.0  # 1024 ** -0.5

F32 = mybir.dt.float32
BF16 = mybir.dt.bfloat16
F8 = mybir.dt.float8e4
DR = mybir.MatmulPerfMode.DoubleRow


def _build_nc():
    nc = bacc.Bacc("TRN2")

    xbT = nc.dram_tensor("xbT", [CIN, T], BF16, kind="ExternalInput").ap()
    wq8 = nc.dram_tensor("wq8", [CIN, DOUT], F8, kind="ExternalInput").ap()
    wk8 = nc.dram_tensor("wk8", [CIN, DOUT], F8, kind="ExternalInput").ap()
    wvT = nc.dram_tensor("wvT", [CIN, DOUT], BF16, kind="ExternalInput").ap()
    cos4 = nc.dram_tensor("cos4", [P, T], BF16, kind="ExternalInput").ap()
    sin4 = nc.dram_tensor("sin4", [P, T], BF16, kind="ExternalInput").ap()
    utri = nc.dram_tensor("utri", [P, P], BF16, kind="ExternalInput").ap()
    outT = nc.dram_tensor("outT", [NHC * (HS + 1), T], F32, kind="ExternalOutput").ap()

    with tile.TileContext(nc) as tc, ExitStack() as ctx:
        const_pool = ctx.enter_context(tc.tile_pool(name="const", bufs=1))
        wpool = ctx.enter_context(tc.tile_pool(name="w", bufs=1))
        xpool = ctx.enter_context(tc.tile_pool(name="x", bufs=1))
        qkpool = ctx.enter_context(tc.tile_pool(name="qk", bufs=1))
        mpool = ctx.enter_context(tc.tile_pool(name="m", bufs=2))
        vpool = ctx.enter_context(tc.tile_pool(name="vaug", bufs=1))
        ppool = ctx.enter_context(tc.tile_pool(name="pt", bufs=2))
        opool = ctx.enter_context(tc.tile_pool(name="ob", bufs=3))

        pp_acc = ctx.enter_context(tc.tile_pool(name="pp_acc", bufs=2, space="PSUM"))
        pp_s = ctx.enter_context(tc.tile_pool(name="pp_s", bufs=2, space="PSUM"))
        pp_po = ctx.enter_context(tc.tile_pool(name="pp_po", bufs=2, space="PSUM"))

        # ---- input DMAs. sync ring: trig first (rope needs it early), then
        # weights, then the rope swap DMAs, then outputs. gpsimd ring: the
        # four x chunks. scalar ring carries only casts+exps (any DMA there
        # would head-of-line-block softmax).
        cos_s = const_pool.tile([P, T], BF16, tag="cos")
        nc.sync.dma_start(cos_s[:], cos4)
        sin_s = const_pool.tile([P, T], BF16, tag="sin")
        nc.sync.dma_start(sin_s[:], sin4)
        w_tiles = {}
        for name, wsrc, dt in (("q", wq8, F8), ("k", wk8, F8), ("v", wvT, BF16)):
            w_s = wpool.tile([P, NCT * DOUT], dt, tag=f"w{name}", name=f"w{name}")
            nc.sync.dma_start(
                w_s.rearrange("p (n d) -> p n d", n=NCT),
                wsrc.rearrange("(n p) d -> p n d", p=P),
            )
            w_tiles[name] = w_s
        utri_s = const_pool.tile([P, P], BF16, tag="utri")
        nc.sync.dma_start(utri_s[:], utri)
        xb_r = xbT.rearrange("(n p) t -> p n t", p=P)
        xsb_t, xs8_t = [], []
        for ch in range(4):
            xt = xpool.tile([P, NCT * 512], BF16, tag=f"xsb{ch}", name=f"xsb{ch}")
            nc.gpsimd.dma_start(
                xt.rearrange("p (n t) -> p n t", n=NCT),
                xb_r[:, :, ch * 512:(ch + 1) * 512],
            )
            xsb_t.append(xt)
            x8 = xpool.tile([P, NCT * 512], F8, tag=f"xs8{ch}", name=f"xs8{ch}")
            xs8_t.append(x8)

        def cast_chunk(ch):
            """bf16 -> fp8 cast of one x chunk, per c-pair so the DR
            projection can start as soon as its own slice is ready."""
            xt_r = xsb_t[ch].rearrange("p (n t) -> p n t", n=NCT)
            x8_r = xs8_t[ch].rearrange("p (n t) -> p n t", n=NCT)
            for cp in range(NCP):
                nc.scalar.copy(
                    x8_r[:, 2 * cp:2 * cp + 2, :], xt_r[:, 2 * cp:2 * cp + 2, :]
                )

        cast_chunk(0)
        cast_chunk(1)  # chunks 2,3 are cast mid-attention (ACT is free then)

        # roped q/k as per-1024-col half tiles (head pair m: rows 0-63/64-127)
        qth = [[qkpool.tile([P, 1024], BF16, tag=f"qt{m}_{h2}", name=f"qt{m}_{h2}")
                for h2 in range(2)] for m in range(2)]
        kth = [[qkpool.tile([P, 1024], BF16, tag=f"kt{m}_{h2}", name=f"kt{m}_{h2}")
                for h2 in range(2)] for m in range(2)]
        va = [
            vpool.tile([P, NHC * (HS + 1)], BF16, tag=f"vaug{tb}", name=f"vaug{tb}")
            for tb in range(T // P)
        ]

        def proj_rope_half(m, wname, dsts, half):
            """fp8 DR projection of one 1024-col half of an m-tile + RoPE."""
            w_r = w_tiles[wname].rearrange("p (n d) -> p n d", n=NCT)
            ra = mpool.tile([P, 1024], BF16, tag="ra", name=f"ra{wname}{m}{half}")
            rp = mpool.tile([P, 1024], BF16, tag="rp", name=f"rp{wname}{m}{half}")
            for chh in range(2):
                ch = half * 2 + chh
                cs = slice(ch * 512, (ch + 1) * 512)
                hs = slice(chh * 512, (chh + 1) * 512)
                x8_r = xs8_t[ch].rearrange("p (n t) -> p n t", n=NCT)
                ps = pp_acc.tile([P, 512], F32, tag="acc", name=f"pj{wname}{m}{ch}")
                for cp in range(NCP):
                    nc.tensor.matmul(
                        ps[:],
                        lhsT=w_r[:, 2 * cp:2 * cp + 2, m * P:(m + 1) * P],
                        rhs=x8_r[:, 2 * cp:2 * cp + 2, :],
                        perf_mode=DR,
                        start=(cp == 0),
                        stop=(cp == NCP - 1),
                    )
                nc.vector.tensor_mul(ra[:, hs], ps[:], cos_s[:, cs])
                nc.vector.tensor_mul(rp[:, hs], ps[:], sin_s[:, cs])
            sw = mpool.tile([P, 1024], BF16, tag="rs", name=f"rs{wname}{m}{half}")
            for blk in range(4):
                s0 = (blk ^ 1) * 32
                nc.sync.dma_start(
                    sw[blk * 32:(blk + 1) * 32, :], rp[s0:s0 + 32, :]
                )
            nc.vector.tensor_sub(dsts[half][:], ra[:], sw[:])

        def vproj(tbp):
            """bf16 V proj of t-blocks (2*tbp, 2*tbp+1) into natural layout."""
            pv = pp_acc.tile([P, 512], F32, tag="acc", name=f"pv{tbp}")
            wv_r = w_tiles["v"].rearrange("p (n d) -> p n d", n=NCT)
            for i in range(2):
                tb = 2 * tbp + i
                xb_c = xsb_t[tb // 4].rearrange("p (n t) -> p n t", n=NCT)
                tb4 = tb % 4
                for c in range(NCT):
                    nc.tensor.matmul(
                        pv[:, i * DOUT:(i + 1) * DOUT],
                        lhsT=xb_c[:, c, tb4 * P:(tb4 + 1) * P],
                        rhs=wv_r[:, c, :],
                        start=(c == 0),
                        stop=(c == NCT - 1),
                        skip_group_check=True,
                    )
            pv_r = pv.rearrange("p (i h d) -> p i h d", i=2, h=NHC)
            for i in range(2):
                vt_r = va[2 * tbp + i].rearrange("p (h e) -> p h e", e=HS + 1)
                nc.gpsimd.memset(vt_r[:, :, HS:HS + 1], 1.0)
                nc.vector.tensor_copy(vt_r[:, :, 0:HS], pv_r[:, i, :, :])

        def pv_finish(m, hi, qc, po):
            h = 2 * m + hi
            q0 = qc * 512
            ob = opool.tile([HS + 1, 512], F32, tag="ob", name=f"ob{h}_{qc}")
            nc.vector.tensor_copy(ob[:], po[:])
            nc.sync.dma_start(
                outT[h * (HS + 1):(h + 1) * (HS + 1), q0:q0 + 512], ob[:]
            )

        def pv_mm(m, hi, po, entry, i, last):
            j, pt, ptoff, w, col0 = entry
            h = 2 * m + hi
            nc.tensor.matmul(
                po[:, col0:512],
                lhsT=va[j][:, h * (HS + 1):(h + 1) * (HS + 1)],
                rhs=pt[:, ptoff:ptoff + w],
                start=(i == 0),
                stop=last,
                skip_group_check=True,
            )

        def attn_chunk(m, hi, qc, prev_plan):
            """scores+exp+mask for (m,hi,qc), braided with PV matmuls for
            chunk qc-1 (2 per score pair) so the PE always has
            ACT-independent work while exp drains the score PSUM rotation.

            Key blocks are processed in pairs sharing one [128,1024] PSUM
            tile so each exp covers both blocks' valid columns (contiguous
            by construction). Returns the PV plan for this chunk."""
            r0 = hi * HS
            q0 = qc * 512
            qt_h = qth[m][q0 // 1024]
            qq0 = q0 % 1024
            po = pv_i = None
            if prev_plan is not None:
                po = pp_po.tile([HS + 1, 512], F32, tag="po",
                                name=f"po{2 * m + hi}_{qc - 1}")
                pv_i = 0
            plan = []
            for jp in range(2 * qc + 2):
                ps = pp_s.tile([P, 1024], F32, tag="ps", name=f"ps{m}{hi}{qc}{jp}")
                pt = ppool.tile([P, 1024], BF16, tag=f"pt{hi}_{jp}",
                                name=f"pt{m}{hi}{qc}{jp}")
                off = 0
                for j in (2 * jp, 2 * jp + 1):
                    col0 = max(0, j * P - q0)
                    w = 512 - col0
                    kt_h = kth[m][(j * P) // 1024]
                    kk0 = (j * P) % 1024
                    nc.tensor.matmul(
                        ps[:, off:off + w],
                        lhsT=kt_h[r0:r0 + HS, kk0:kk0 + P],
                        rhs=qt_h[r0:r0 + HS, qq0 + col0:qq0 + 512],
                        start=True,
                        stop=True,
                        tile_position=(r0, 0),
                    )
                    plan.append((j, pt, off, w, col0))
                    off += w
                nc.scalar.activation(
                    pt[:, 0:off], ps[:, 0:off],
                    mybir.ActivationFunctionType.Exp, scale=SCALE,
                )
                if prev_plan is not None:
                    for _ in range(2):
                        if pv_i < len(prev_plan):
                            pv_mm(m, hi, po, prev_plan[pv_i], pv_i,
                                  pv_i == len(prev_plan) - 1)
                            pv_i += 1
            for j, pt, ptoff, w, col0 in plan:
                if col0 > 0 or j * P == q0:  # diagonal block: causal mask
                    nc.vector.tensor_mul(
                        pt[:, ptoff:ptoff + P], pt[:, ptoff:ptoff + P], utri_s[:]
                    )
            if prev_plan is not None:
                pv_finish(m, hi, qc - 1, po)
            return plan

        def pv_tail(m, hi, qc, plan):
            po = pp_po.tile([HS + 1, 512], F32, tag="po",
                            name=f"po{2 * m + hi}_{qc}")
            for i, entry in enumerate(plan):
                pv_mm(m, hi, po, entry, i, i == len(plan) - 1)
            pv_finish(m, hi, qc, po)

        # ---- pair-0 first-half projections, then qc-major attention per
        # pair; the second halves, pair-1 projections and V-proj are all
        # issued inside pair-0's loop as PE filler.
        proj_rope_half(0, "q", qth[0], 0)
        proj_rope_half(0, "k", kth[0], 0)

        for m in (0, 1):
            plans = {0: None, 1: None}
            for qc in range(5):
                for hi in range(2):
                    if qc < 4:
                        plans[hi] = attn_chunk(m, hi, qc, plans[hi])
                    else:
                        pv_tail(m, hi, 3, plans[hi])
                    if m == 0 and qc == 0:
                        cast_chunk(2 + hi)
                if m == 0:
                    if qc == 0:
                        vproj(0)
                        vproj(1)
                    elif qc == 1:
                        proj_rope_half(0, "q", qth[0], 1)
                        proj_rope_half(0, "k", kth[0], 1)
                        vproj(2)
                        vproj(3)
                    elif qc == 2:
                        proj_rope_half(1, "q", qth[1], 0)
                        proj_rope_half(1, "k", kth[1], 0)
                        proj_rope_half(1, "q", qth[1], 1)
                        proj_rope_half(1, "k", kth[1], 1)
                        for tbp in (4, 5, 6, 7):
                            vproj(tbp)

    nc.compile()
    return nc


_CACHE = {}


def _get_nc():
    if "nc" not in _CACHE:
        _CACHE["nc"] = _build_nc()
    return _CACHE["nc"]


def _host_inputs(x, Wq, Wk, Wv):
    bf = ml_dtypes.bfloat16
    f8 = ml_dtypes.float8_e4m3
    B = x.shape[0]
    # RoPE tables (match reference: theta over hs/2 freqs with dim=n_emb)
    i = np.arange(HS // 2, dtype=np.float32)
    theta = np.float32(10000.0) ** (-2.0 * i / np.float32(CIN))
    pos = np.arange(T, dtype=np.float32)
    ang = pos[:, None] * theta[None, :]
    cosT = np.cos(ang).T.astype(np.float32)  # [32, T]
    sinT = np.sin(ang).T.astype(np.float32)
    cos4 = np.ascontiguousarray(np.tile(cosT, (4, 1))).astype(bf)
    sin4 = np.ascontiguousarray(
        np.tile(np.concatenate([-sinT, sinT], axis=0), (2, 1))
    ).astype(bf)  # rows: [-sin, +sin] x2
    utri_np = np.triu(np.ones((P, P), np.float32)).astype(bf)

    perm = np.concatenate([np.arange(0, HS, 2), np.arange(1, HS, 2)])
    in_maps = []
    for core in range(8):
        b, g = core // 4, core % 4
        idx = np.concatenate([(4 * g + h) * HS + perm for h in range(NHC)])
        xT = np.ascontiguousarray(x[b].T)
        m = {
            "xbT": xT.astype(bf),
            "wq8": np.ascontiguousarray(Wq[idx].T).astype(f8),
            "wk8": np.ascontiguousarray(Wk[idx].T).astype(f8),
            "wvT": np.ascontiguousarray(Wv[g * DOUT:(g + 1) * DOUT].T).astype(bf),
            "cos4": cos4,
            "sin4": sin4,
            "utri": utri_np,
        }
        in_maps.append(m)
    return in_maps


def kernel(x, Wq, Wk, Wv, _trace=False, _trace_kwargs=None):
    x = np.asarray(x)
    Wq, Wk, Wv = np.asarray(Wq), np.asarray(Wk), np.asarray(Wv)
    B = x.shape[0]
    nc = _get_nc()
    in_maps = _host_inputs(x, Wq, Wk, Wv)
    res = run_bass_kernel_spmd(
        nc, in_maps, list(range(8)), trace=_trace, **(_trace_kwargs or {})
    )
    out = np.zeros((B, T, CIN), np.float32)
    for core in range(8):
        b, g = core // 4, core % 4
        r = res.results[core]["outT"].reshape(NHC, HS + 1, T)
        o = r[:, 0:HS, :] / r[:, HS:HS + 1, :]
        out[b, :, g * DOUT:(g + 1) * DOUT] = o.reshape(DOUT, T).T
    if _trace:
        return out, res
    return out


# revision 45
# speedup vs baseline: 1.1815x; 1.1469x over previous
"""Causal attention head (RoPE) kernel for 8 Trainium2 NeuronCores.

Sharding: 8 cores = 2 batches x 4 head-groups (4 heads each), no
cross-device comms. Per-core design (v12):

  - ONE bf16 x load (4 per-512-t-chunk tiles, gpsimd DMA ring); the fp8
    copy for the q/k projections is cast chunk-by-chunk on the ACT engine
    (idle before softmax), so the fp8 path starts as each chunk lands.
  - Q/K projections as fp8 DoubleRow matmuls (K_eff=256/instr); RoPE per
    1024-col half: cos/sin products on DVE (bf16), partition swap via 4
    sync-ring DMAs, one wide DVE sub -> roped bf16 half-tiles. Scores over
    the first q/k half can start while the second half is still roping.
  - V projected straight into natural (key-major) layout with x^T bf16
    stationary (no PE transposes); ones column per head makes PV row 64
    the softmax denominator. fp8 anywhere in the P/V path provably
    exceeds the error budget, so it stays bf16.
  - attention is qc-major (per 512-query chunk, all key blocks up to the
    diagonal), scores bf16 dual-tile (two heads on PE row-tiles 0-63 /
    64-127, K=64). Score PSUM tiles hold PAIRS of key blocks so exp runs
    >=384 wide (exp has ~700ns fixed cost); diagonal blocks use
    partial-width matmuls. PV for chunk qc is issued during chunk qc+1's
    scores so the PE never waits on exp (PE p-state: any gap halves the
    clock); pair-1 projections and V-proj fill remaining PE slack.
  - po [65, 512] PSUM -> DVE copy -> HBM unnormalized; the host divides
    by the denominator row on gather (free).
"""

import os
import sys
from contextlib import ExitStack

import numpy as np

for _p in ("/opt/trn_rl_repo", "/root/.axon_site/_ro/trn_rl_repo"):
    if os.path.isdir(_p) and _p not in sys.path:
        sys.path.append(_p)

import ml_dtypes

import concourse.bass as bass
import concourse.mybir as mybir
import concourse.tile as tile
from concourse import bacc
from concourse.bass_utils import run_bass_kernel_spmd

P = 128
T = 2048
CIN = 1024
NHC = 4          # heads per core
HS = 64
DOUT = NHC * HS  # 256
NCT = CIN // P   # 8 contraction tiles
NCP = NCT // 2   # 4 fp8 DoubleRow contraction pairs
SCALE = 1.0 / 32.0  # 1024 ** -0.5

F32 = mybir.dt.float32
BF16 = mybir.dt.bfloat16
F8 = mybir.dt.float8e4
DR = mybir.MatmulPerfMode.DoubleRow


def _build_nc():
    nc = bacc.Bacc("TRN2")

    xbT = nc.dram_tensor("xbT", [CIN, T], BF16, kind="ExternalInput").ap()
    wq8 = nc.dram_tensor("wq8", [CIN, DOUT], F8, kind="ExternalInput").ap()
    wk8 = nc.dram_tensor("wk8", [CIN, DOUT], F8, kind="ExternalInput").ap()
    wvT = nc.dram_tensor("wvT", [CIN, DOUT], BF16, kind="ExternalInput").ap()
    cos4 = nc.dram_tensor("cos4", [P, T], BF16, kind="ExternalInput").ap()
    sin4 = nc.dram_tensor("sin4", [P, T], BF16, kind="ExternalInput").ap()
    utri = nc.dram_tensor("utri", [P, P], BF16, kind="ExternalInput").ap()
    outT = nc.dram_tensor("outT", [NHC * (HS + 1), T], F32, kind="ExternalOutput").ap()

    with tile.TileContext(nc) as tc, ExitStack() as ctx:
        const_pool = ctx.enter_context(tc.tile_pool(name="const", bufs=1))
        wpool = ctx.enter_context(tc.tile_pool(name="w", bufs=1))
        xpool = ctx.enter_context(tc.tile_pool(name="x", bufs=1))
        qkpool = ctx.enter_context(tc.tile_pool(name="qk", bufs=1))
        mpool = ctx.enter_context(tc.tile_pool(name="m", bufs=2))
        vpool = ctx.enter_context(tc.tile_pool(name="vaug", bufs=1))
        ppool = ctx.enter_context(tc.tile_pool(name="pt", bufs=2))
        opool = ctx.enter_context(tc.tile_pool(name="ob", bufs=3))

        pp_acc = ctx.enter_context(tc.tile_pool(name="pp_acc", bufs=1, space="PSUM"))
        pp_s = ctx.enter_context(tc.tile_pool(name="pp_s", bufs=3, space="PSUM"))
        pp_po = ctx.enter_context(tc.tile_pool(name="pp_po", bufs=1, space="PSUM"))

        # ---- input DMAs. sync ring: trig first (rope needs it early), then
        # weights, then the rope swap DMAs, then outputs. gpsimd ring: the
        # four x chunks. scalar ring carries only casts+exps (any DMA there
        # would head-of-line-block softmax).
        w_tiles = {}
        for name, wsrc, dt in (("q", wq8, F8), ("k", wk8, F8)):
            w_s = wpool.tile([P, NCT * DOUT], dt, tag=f"w{name}", name=f"w{name}")
            nc.sync.dma_start(
                w_s.rearrange("p (n d) -> p n d", n=NCT),
                wsrc.rearrange("(n p) d -> p n d", p=P),
            )
            w_tiles[name] = w_s
        cos_s = const_pool.tile([P, T], BF16, tag="cos")
        nc.sync.dma_start(cos_s[:], cos4)
        sin_s = const_pool.tile([P, T], BF16, tag="sin")
        nc.sync.dma_start(sin_s[:], sin4)
        wv_s = wpool.tile([P, NCT * DOUT], BF16, tag="wv", name="wv")
        nc.sync.dma_start(
            wv_s.rearrange("p (n d) -> p n d", n=NCT),
            wvT.rearrange("(n p) d -> p n d", p=P),
        )
        w_tiles["v"] = wv_s
        utri_s = const_pool.tile([P, P], BF16, tag="utri")
        nc.sync.dma_start(utri_s[:], utri)
        xb_r = xbT.rearrange("(n p) t -> p n t", p=P)
        xsb_t, xs8_t = [], []
        for ch in range(4):
            xt = xpool.tile([P, NCT * 512], BF16, tag=f"xsb{ch}", name=f"xsb{ch}")
            nc.gpsimd.dma_start(
                xt.rearrange("p (n t) -> p n t", n=NCT),
                xb_r[:, :, ch * 512:(ch + 1) * 512],
            )
            xsb_t.append(xt)
            x8 = xpool.tile([P, NCT * 512], F8, tag=f"xs8{ch}", name=f"xs8{ch}")
            xs8_t.append(x8)

        def cast_chunk(ch):
            """bf16 -> fp8 cast of one x chunk, per c-pair so the DR
            projection can start as soon as its own slice is ready."""
            xt_r = xsb_t[ch].rearrange("p (n t) -> p n t", n=NCT)
            x8_r = xs8_t[ch].rearrange("p (n t) -> p n t", n=NCT)
            for cp in range(NCP):
                nc.scalar.copy(
                    x8_r[:, 2 * cp:2 * cp + 2, :], xt_r[:, 2 * cp:2 * cp + 2, :]
                )

        cast_chunk(0)
        cast_chunk(1)  # chunks 2,3 are cast mid-attention (ACT is free then)

        # roped q/k as per-1024-col half tiles (head pair m: rows 0-63/64-127)
        qth = [[qkpool.tile([P, 1024], BF16, tag=f"qt{m}_{h2}", name=f"qt{m}_{h2}")
                for h2 in range(2)] for m in range(2)]
        kth = [[qkpool.tile([P, 1024], BF16, tag=f"kt{m}_{h2}", name=f"kt{m}_{h2}")
                for h2 in range(2)] for m in range(2)]
        va = [
            vpool.tile([P, NHC * (HS + 1)], BF16, tag=f"vaug{tb}", name=f"vaug{tb}")
            for tb in range(T // P)
        ]

        def proj_rope_half(m, wname, dsts, half):
            """fp8 DR projection of one 1024-col half of an m-tile + RoPE."""
            w_r = w_tiles[wname].rearrange("p (n d) -> p n d", n=NCT)
            ra = mpool.tile([P, 1024], BF16, tag="ra", name=f"ra{wname}{m}{half}")
            rp = mpool.tile([P, 1024], BF16, tag="rp", name=f"rp{wname}{m}{half}")
            for chh in range(2):
                ch = half * 2 + chh
                cs = slice(ch * 512, (ch + 1) * 512)
                hs = slice(chh * 512, (chh + 1) * 512)
                x8_r = xs8_t[ch].rearrange("p (n t) -> p n t", n=NCT)
                ps = pp_acc.tile([P, 512], F32, tag="acc", name=f"pj{wname}{m}{ch}")
                for cp in range(NCP):
                    nc.tensor.matmul(
                        ps[:],
                        lhsT=w_r[:, 2 * cp:2 * cp + 2, m * P:(m + 1) * P],
                        rhs=x8_r[:, 2 * cp:2 * cp + 2, :],
                        perf_mode=DR,
                        start=(cp == 0),
                        stop=(cp == NCP - 1),
                    )
                nc.vector.tensor_mul(ra[:, hs], ps[:], cos_s[:, cs])
                nc.vector.tensor_mul(rp[:, hs], ps[:], sin_s[:, cs])
            sw = mpool.tile([P, 1024], BF16, tag="rs", name=f"rs{wname}{m}{half}")
            for blk in range(4):
                s0 = (blk ^ 1) * 32
                nc.sync.dma_start(
                    sw[blk * 32:(blk + 1) * 32, :], rp[s0:s0 + 32, :]
                )
            nc.vector.tensor_sub(dsts[half][:], ra[:], sw[:])

        def vproj(tbp):
            """bf16 V proj of t-blocks (2*tbp, 2*tbp+1) into natural layout."""
            pv = pp_acc.tile([P, 512], F32, tag="acc", name=f"pv{tbp}")
            wv_r = w_tiles["v"].rearrange("p (n d) -> p n d", n=NCT)
            for i in range(2):
                tb = 2 * tbp + i
                xb_c = xsb_t[tb // 4].rearrange("p (n t) -> p n t", n=NCT)
                tb4 = tb % 4
                for c in range(NCT):
                    nc.tensor.matmul(
                        pv[:, i * DOUT:(i + 1) * DOUT],
                        lhsT=xb_c[:, c, tb4 * P:(tb4 + 1) * P],
                        rhs=wv_r[:, c, :],
                        start=(c == 0),
                        stop=(c == NCT - 1),
                        skip_group_check=True,
                    )
            pv_r = pv.rearrange("p (i h d) -> p i h d", i=2, h=NHC)
            for i in range(2):
                vt_r = va[2 * tbp + i].rearrange("p (h e) -> p h e", e=HS + 1)
                nc.gpsimd.memset(vt_r[:, :, HS:HS + 1], 1.0)
                nc.vector.tensor_copy(vt_r[:, :, 0:HS], pv_r[:, i, :, :])

        def pv_finish(m, hi, qc, po):
            h = 2 * m + hi
            q0 = qc * 512
            ob = opool.tile([HS + 1, 512], F32, tag="ob", name=f"ob{h}_{qc}")
            nc.vector.tensor_copy(ob[:], po[:])
            nc.sync.dma_start(
                outT[h * (HS + 1):(h + 1) * (HS + 1), q0:q0 + 512], ob[:]
            )

        def pv_mm(m, hi, po, entry, i, last):
            j, pt, ptoff, w, col0 = entry
            h = 2 * m + hi
            nc.tensor.matmul(
                po[:, col0:512],
                lhsT=va[j][:, h * (HS + 1):(h + 1) * (HS + 1)],
                rhs=pt[:, ptoff:ptoff + w],
                start=(i == 0),
                stop=last,
                skip_group_check=True,
            )

        def attn_chunk(m, hi, qc, prev_plan):
            """scores+exp+mask for (m,hi,qc), braided with PV matmuls for
            chunk qc-1 (2 per score pair) so the PE always has
            ACT-independent work while exp drains the score PSUM rotation.

            Key blocks are processed in pairs sharing one [128,1024] PSUM
            tile so each exp covers both blocks' valid columns (contiguous
            by construction). Returns the PV plan for this chunk."""
            r0 = hi * HS
            q0 = qc * 512
            qt_h = qth[m][q0 // 1024]
            qq0 = q0 % 1024
            po = pv_i = None
            if prev_plan is not None:
                po = pp_po.tile([HS + 1, 512], F32, tag="po",
                                name=f"po{2 * m + hi}_{qc - 1}")
                pv_i = 0
            plan = []
            for jp in range(2 * qc + 2):
                ps = pp_s.tile([P, 1024], F32, tag="ps", name=f"ps{m}{hi}{qc}{jp}")
                pt = ppool.tile([P, 1024], BF16, tag=f"pt{hi}_{jp}",
                                name=f"pt{m}{hi}{qc}{jp}")
                off = 0
                for j in (2 * jp, 2 * jp + 1):
                    col0 = max(0, j * P - q0)
                    w = 512 - col0
                    kt_h = kth[m][(j * P) // 1024]
                    kk0 = (j * P) % 1024
                    nc.tensor.matmul(
                        ps[:, off:off + w],
                        lhsT=kt_h[r0:r0 + HS, kk0:kk0 + P],
                        rhs=qt_h[r0:r0 + HS, qq0 + col0:qq0 + 512],
                        start=True,
                        stop=True,
                        tile_position=(r0, 0),
                    )
                    plan.append((j, pt, off, w, col0))
                    off += w
                nc.scalar.activation(
                    pt[:, 0:off], ps[:, 0:off],
                    mybir.ActivationFunctionType.Exp, scale=SCALE,
                )
                if prev_plan is not None:
                    for _ in range(2):
                        if pv_i < len(prev_plan):
                            pv_mm(m, hi, po, prev_plan[pv_i], pv_i,
                                  pv_i == len(prev_plan) - 1)
                            pv_i += 1
            for j, pt, ptoff, w, col0 in plan:
                if col0 > 0 or j * P == q0:  # diagonal block: causal mask
                    nc.vector.tensor_mul(
                        pt[:, ptoff:ptoff + P], pt[:, ptoff:ptoff + P], utri_s[:]
                    )
            if prev_plan is not None:
                pv_finish(m, hi, qc - 1, po)
            return plan

        def pv_tail(m, hi, qc, plan):
            po = pp_po.tile([HS + 1, 512], F32, tag="po",
                            name=f"po{2 * m + hi}_{qc}")
            for i, entry in enumerate(plan):
                pv_mm(m, hi, po, entry, i, i == len(plan) - 1)
            pv_finish(m, hi, qc, po)

        # ---- pair-0 first-half projections, then qc-major attention per
        # pair; the second halves, pair-1 projections and V-proj are all
        # issued inside pair-0's loop as PE filler.
        proj_rope_half(0, "q", qth[0], 0)
        proj_rope_half(0, "k", kth[0], 0)

        for m in (0, 1):
            plans = {0: None, 1: None}
            for qc in range(5):
                for hi in range(2):
                    if qc < 4:
                        plans[hi] = attn_chunk(m, hi, qc, plans[hi])
                    else:
                        pv_tail(m, hi, 3, plans[hi])
                    if m == 0 and qc == 0:
                        cast_chunk(2 + hi)
                if m == 0:
                    if qc == 0:
                        vproj(0)
                        vproj(1)
                    elif qc == 1:
                        proj_rope_half(0, "q", qth[0], 1)
                        proj_rope_half(0, "k", kth[0], 1)
                        vproj(2)
                        vproj(3)
                    elif qc == 2:
                        proj_rope_half(1, "q", qth[1], 0)
                        proj_rope_half(1, "k", kth[1], 0)
                        proj_rope_half(1, "q", qth[1], 1)
                        proj_rope_half(1, "k", kth[1], 1)
                        for tbp in (4, 5, 6, 7):
                            vproj(tbp)

    nc.compile()
    return nc


_CACHE = {}


def _get_nc():
    if "nc" not in _CACHE:
        _CACHE["nc"] = _build_nc()
    return _CACHE["nc"]


def _host_inputs(x, Wq, Wk, Wv):
    bf = ml_dtypes.bfloat16
    f8 = ml_dtypes.float8_e4m3
    B = x.shape[0]
    # RoPE tables (match reference: theta over hs/2 freqs with dim=n_emb)
    i = np.arange(HS // 2, dtype=np.float32)
    theta = np.float32(10000.0) ** (-2.0 * i / np.float32(CIN))
    pos = np.arange(T, dtype=np.float32)
    ang = pos[:, None] * theta[None, :]
    cosT = np.cos(ang).T.astype(np.float32)  # [32, T]
    sinT = np.sin(ang).T.astype(np.float32)
    cos4 = np.ascontiguousarray(np.tile(cosT, (4, 1))).astype(bf)
    sin4 = np.ascontiguousarray(
        np.tile(np.concatenate([-sinT, sinT], axis=0), (2, 1))
    ).astype(bf)  # rows: [-sin, +sin] x2
    utri_np = np.triu(np.ones((P, P), np.float32)).astype(bf)

    perm = np.concatenate([np.arange(0, HS, 2), np.arange(1, HS, 2)])
    in_maps = []
    for core in range(8):
        b, g = core // 4, core % 4
        idx = np.concatenate([(4 * g + h) * HS + perm for h in range(NHC)])
        xT = np.ascontiguousarray(x[b].T)
        m = {
            "xbT": xT.astype(bf),
            "wq8": np.ascontiguousarray(Wq[idx].T).astype(f8),
            "wk8": np.ascontiguousarray(Wk[idx].T).astype(f8),
            "wvT": np.ascontiguousarray(Wv[g * DOUT:(g + 1) * DOUT].T).astype(bf),
            "cos4": cos4,
            "sin4": sin4,
            "utri": utri_np,
        }
        in_maps.append(m)
    return in_maps


def kernel(x, Wq, Wk, Wv, _trace=False, _trace_kwargs=None):
    x = np.asarray(x)
    Wq, Wk, Wv = np.asarray(Wq), np.asarray(Wk), np.asarray(Wv)
    B = x.shape[0]
    nc = _get_nc()
    in_maps = _host_inputs(x, Wq, Wk, Wv)
    res = run_bass_kernel_spmd(
        nc, in_maps, list(range(8)), trace=_trace, **(_trace_kwargs or {})
    )
    out = np.zeros((B, T, CIN), np.float32)
    for core in range(8):
        b, g = core // 4, core % 4
        r = res.results[core]["outT"].reshape(NHC, HS + 1, T)
        o = r[:, 0:HS, :] / r[:, HS:HS + 1, :]
        out[b, :, g * DOUT:(g + 1) * DOUT] = o.reshape(DOUT, T).T
    if _trace:
        return out, res
    return out
